# revision 38
# baseline (speedup 1.0000x reference)
"""DeepSet cell encoder on 8 Trainium2 NeuronCores.

Strategy: sort edges by cell, split cells into 8 contiguous ranges with
~equal edge counts (one range per core -> no collectives). Each core:
  - batched dma_gather of node features (bf16, phase-compacted tables so
    indices fit int16)
  - phi MLP: layer1 bf16 (PE-transposed x), layer2 fp32r
  - segment-sum via per-tile one-hot matmul accumulated in PSUM per
    128-cell block
  - rho MLP in transposed layout, fp32r, N=512
Output written transposed [256, cells]; host unpacks to [M, 256].
"""
import sys

if "/opt/trn_rl_repo" not in sys.path:
    sys.path.insert(0, "/opt/trn_rl_repo")

import numpy as np
import ml_dtypes

import concourse.bass as bass
import concourse.mybir as mybir
import concourse.tile as tile
from concourse import bacc
from concourse.bass_utils import run_bass_kernel_spmd
from concourse.masks import make_identity

P = 128
N_NODES, E_TOT, M_CELLS = 100000, 400000, 50000
D_IN, D_H, D_OUT = 384, 256, 256
NCORES = 8
TPB = 8            # tiles per block
BLK_EDGES = TPB * P   # 1024 edge slots per block
NPH = 2            # index phases (int16 range)

f32 = mybir.dt.float32
f32r = mybir.dt.float32r
bf16 = mybir.dt.bfloat16
i16 = mybir.dt.int16
RELU = mybir.ActivationFunctionType.Relu


# ----------------------------------------------------------------- host prep
def _prepare(chunk_features, flat_nodes, cell_asgn):
    x_bf = chunk_features.astype(ml_dtypes.bfloat16)
    cells = np.asarray(cell_asgn, dtype=np.int64)
    nodes = np.asarray(flat_nodes, dtype=np.int64)
    E = cells.shape[0]

    order = np.argsort(cells, kind="stable")
    sc = cells[order]
    sn = nodes[order]
    counts = np.bincount(cells, minlength=M_CELLS)
    cum = np.concatenate([[0], np.cumsum(counts)])  # cum[m] = edges before cell m

    # split cells into NCORES contiguous ranges with ~equal edges
    targets = [E * k // NCORES for k in range(1, NCORES)]
    splits = [0] + [int(np.searchsorted(cum, t)) for t in targets] + [M_CELLS]

    cores = []
    max_blocks = 0
    for c in range(NCORES):
        m_lo, m_hi = splits[c], splits[c + 1]
        blocks = []  # (first_cell, ncells, e_start, e_end)
        m = m_lo
        while m < m_hi:
            first = m
            e0 = cum[m]
            nc_in = 0
            while m < m_hi and nc_in < P and (cum[m + 1] - e0) <= BLK_EDGES:
                m += 1
                nc_in += 1
            if nc_in == 0:
                raise ValueError(f"cell {m} has {counts[m]} edges > {BLK_EDGES}")
            blocks.append((first, nc_in, int(e0), int(cum[m])))
        cores.append({"m_lo": m_lo, "m_hi": m_hi, "blocks": blocks})
        max_blocks = max(max_blocks, len(blocks))

    B = ((max_blocks + 3) // 4) * 4  # multiple of 4 for rho groups
    T = B * TPB

    # per-core slot arrays + phase tables
    ph_split = (B // 2 + 1) // 2 * 2  # blocks in phase 0 (even)
    ph_split = B // 2
    data = []
    nrows = 0
    for c in range(NCORES):
        blocks = cores[c]["blocks"]
        slot_node = np.zeros((T * P,), np.int64)   # node id per slot (pad -> 0)
        slot_ccol = np.full((T * P,), -1.0, np.float32)
        out_map = []  # (block, first_cell, ncells)
        for b in range(B):
            if b < len(blocks):
                first, nc_in, e0, e1 = blocks[b]
                n_e = e1 - e0
                s0 = b * BLK_EDGES
                slot_node[s0:s0 + n_e] = sn[e0:e1]
                slot_ccol[s0:s0 + n_e] = (sc[e0:e1] - first).astype(np.float32)
                out_map.append((b, first, nc_in))
            # else: empty pad block
        # phases
        ph_of_block = np.arange(B) >= ph_split
        idx16 = np.zeros((T * P,), np.int16)
        uniqs = []
        for ph in range(NPH):
            blk_sel = np.where(ph_of_block == bool(ph))[0]
            slots = np.concatenate([np.arange(b * BLK_EDGES, (b + 1) * BLK_EDGES)
                                    for b in blk_sel])
            nd = slot_node[slots]
            real = nd >= 0
            u, inv = np.unique(nd[real], return_inverse=True)
            if len(u) == 0:
                u = np.array([0], np.int64)
            loc = np.zeros(len(slots), np.int64)
            loc[real] = inv
            idx16[slots] = loc.astype(np.int16)
            uniqs.append(u)
            nrows = max(nrows, len(u))
        data.append({"idx16": idx16, "ccol": slot_ccol, "uniqs": uniqs,
                     "out_map": out_map})
    assert nrows <= 32000, f"phase unique rows {nrows} exceed int16 range"
    NROWS = nrows

    # build final per-core input arrays
    in_maps = []
    for c in range(NCORES):
        d = data[c]
        tabs = []
        for ph in range(NPH):
            t = np.zeros((NROWS, D_IN), ml_dtypes.bfloat16)
            u = d["uniqs"][ph]
            t[:len(u)] = x_bf[u]
            tabs.append(t)
        # idx wrap per gather op (= per block): k = t_in_block*128 + p
        # value at [k%16 + 16*rep, k//16] ; 64 cols per block
        idx_blocks = d["idx16"].reshape(B, TPB * P)
        wrap = np.zeros((P, B * 64), np.int16)
        for b in range(B):
            w16 = idx_blocks[b].reshape(64, 16).T  # [16, 64]
            wrap[:, b * 64:(b + 1) * 64] = np.tile(w16, (8, 1))
        ccol = d["ccol"].reshape(T, P).T.copy()  # [128, T] col t = tile t
        in_maps.append({"tab0": tabs[0], "tab1": tabs[1], "gidx": wrap,
                        "gidx0": np.ascontiguousarray(wrap[:, :4 * 64]),
                        "ccol": np.ascontiguousarray(ccol)})
    meta = {"B": B, "T": T, "NROWS": NROWS, "ph_split": ph_split,
            "out_maps": [d["out_map"] for d in data],
            "m_ranges": [(cores[c]["m_lo"], cores[c]["m_hi"]) for c in range(NCORES)]}
    return in_maps, meta


# ------------------------------------------------------------- device build
def _build(B, NROWS, ph_split, weights_np):
    T = B * TPB
    nc = bacc.Bacc()
    tabs = [nc.dram_tensor(f"tab{p}", [NROWS, D_IN], bf16, kind="ExternalInput")
            for p in range(NPH)]
    gidx = nc.dram_tensor("gidx", [P, B * 64], i16, kind="ExternalInput")
    gidx0 = nc.dram_tensor("gidx0", [P, 4 * 64], i16, kind="ExternalInput")
    ccol = nc.dram_tensor("ccol", [P, T], f32, kind="ExternalInput")
    w1 = nc.dram_tensor("w1", [D_IN, D_H], bf16, kind="ExternalInput")
    w2 = nc.dram_tensor("w2", [D_H, D_H], f32r, kind="ExternalInput")
    r1 = nc.dram_tensor("r1", [D_H, D_H], f32r, kind="ExternalInput")
    r2 = nc.dram_tensor("r2", [D_H, D_H], f32r, kind="ExternalInput")
    r3 = nc.dram_tensor("r3", [D_H, D_OUT], f32r, kind="ExternalInput")
    b1d = nc.dram_tensor("b1", [P, 2], f32, kind="ExternalInput")
    c1d = nc.dram_tensor("c1", [P, 2], f32, kind="ExternalInput")
    c2d = nc.dram_tensor("c2", [P, 2], f32, kind="ExternalInput")
    c3d = nc.dram_tensor("c3", [P, 2], f32, kind="ExternalInput")
    iota = nc.dram_tensor("iota", [P, P], f32, kind="ExternalInput")
    out = nc.dram_tensor("out", [D_OUT, B * P], f32, kind="ExternalOutput")

    b2_nonzero = bool(np.any(weights_np["b2"] != 0.0))
    b2d = None
    if b2_nonzero:
        b2d = nc.dram_tensor("b2row", [P, D_H], f32, kind="ExternalInput")

    with tile.TileContext(nc) as tc:
        with (
            tc.tile_pool(name="const", bufs=1) as cpool,
            tc.tile_pool(name="xg", bufs=3) as xgp,
            tc.tile_pool(name="work", bufs=3) as wp,
            tc.tile_pool(name="csb", bufs=6) as csp,
            tc.tile_pool(name="h2p", bufs=6) as h2p,
            tc.tile_pool(name="ps_big", bufs=4, space="PSUM") as ps_big,
            tc.tile_pool(name="ps_h2", bufs=2, space="PSUM") as ps_h2,
            tc.tile_pool(name="ps_cs", bufs=2, space="PSUM") as ps_cs,
        ):
            # ---- constants
            iota_t = cpool.tile([P, P], f32)
            nc.sync.dma_start(iota_t[:], iota[:])
            gidx0_t = cpool.tile([P, 4 * 64], i16)
            nc.sync.dma_start(gidx0_t[:], gidx0[:])
            gidx_t = cpool.tile([P, B * 64], i16)
            nc.sync.dma_start(gidx_t[:], gidx[:])
            ccol_t = cpool.tile([P, T], f32)
            nc.sync.dma_start(ccol_t[:], ccol[:])
            w1_t = cpool.tile([P, 3, 2, P], bf16)
            for k in range(3):
                for m in range(2):
                    nc.sync.dma_start(w1_t[:, k, m, :],
                                      w1[k * P:(k + 1) * P, m * P:(m + 1) * P])
            w2_t = cpool.tile([P, 2, D_H], f32r)
            for k in range(2):
                nc.sync.dma_start(w2_t[:, k, :], w2[k * P:(k + 1) * P, :])
            rts = []
            for name, drt in (("r1", r1), ("r2", r2), ("r3", r3)):
                rt = cpool.tile([P, 2, 2, P], f32r, tag=f"{name}t")
                for k in range(2):
                    for m in range(2):
                        nc.sync.dma_start(rt[:, k, m, :],
                                          drt[k * P:(k + 1) * P, m * P:(m + 1) * P])
                rts.append(rt)
            r1_t, r2_t, r3_t = rts
            b1_t = cpool.tile([P, 2], f32)
            nc.sync.dma_start(b1_t[:], b1d[:])
            c1_t = cpool.tile([P, 2], f32)
            nc.sync.dma_start(c1_t[:], c1d[:])
            c2_t = cpool.tile([P, 2], f32)
            nc.sync.dma_start(c2_t[:], c2d[:])
            c3_t = cpool.tile([P, 2], f32)
            nc.sync.dma_start(c3_t[:], c3d[:])
            b2_t = None
            if b2_nonzero:
                b2_t = cpool.tile([P, D_H], f32)
                nc.sync.dma_start(b2_t[:], b2d[:])

            cs_ps_tiles = {}
            csT_tiles = {}
            xt_tiles = {}

            def issue_gather(b):
                t = xgp.tile([P, 3, BLK_EDGES], bf16, tag="xg")
                nc.gpsimd.dma_gather(
                    out_ap=t[:], in_ap=(tabs[0] if b < ph_split else tabs[1])[:],
                    idxs_ap=(gidx0_t[:, b * 64:(b + 1) * 64] if b < 4
                             else gidx_t[:, b * 64:(b + 1) * 64]),
                    num_idxs=BLK_EDGES, num_idxs_reg=BLK_EDGES,
                    elem_size=D_IN, single_packet=False, transpose=True,
                )
                return t

            def emit_rho(grp, rin):
                rin_slice = lambda t, k: t[:, k, :, :] if len(t.shape) == 4 else t[:, k, :]
                for li, (rt, cb) in enumerate(((r1_t, c1_t), (r2_t, c2_t))):
                    rout = wp.tile([P, 2, 4 * P], f32r, tag=f"r{li}o")
                    for m in range(2):
                        r_ps = ps_big.tile([P, 4 * P], f32, space="PSUM", tag="big")
                        for k in range(2):
                            nc.tensor.matmul(r_ps[:], lhsT=rt[:, k, m, :],
                                             rhs=rin_slice(rin, k),
                                             start=(k == 0), stop=(k == 1))
                        nc.scalar.activation(out=rout[:, m, :], in_=r_ps[:],
                                             func=RELU, bias=cb[:, m:m + 1])
                    rin = rout
                for m in range(2):
                    r_ps = ps_big.tile([P, 4 * P], f32, space="PSUM", tag="big")
                    for k in range(2):
                        nc.tensor.matmul(r_ps[:], lhsT=r3_t[:, k, m, :],
                                         rhs=rin_slice(rin, k),
                                         start=(k == 0), stop=(k == 1))
                    o_sb = wp.tile([P, 4 * P], f32, tag="o_sb")
                    nc.vector.tensor_scalar(out=o_sb[:], in0=r_ps[:],
                                            scalar1=c3_t[:, m:m + 1], scalar2=None,
                                            op0=mybir.AluOpType.add)
                    nc.sync.dma_start(
                        out[m * P:(m + 1) * P, grp * 4 * P:(grp + 1) * 4 * P],
                        o_sb[:])

            for blk in range(B):
                xt_all = issue_gather(blk)
                # csT accumulator: [fchunk, f128, cells] transposed segment sums
                cs_ps = ps_cs.tile([P, 2, P], f32, space="PSUM", tag="cs")
                cs_ps_tiles[blk] = cs_ps
                if blk % 4 == 0:
                    csT_grp = wp.tile([P, 2, 4, P], f32r, tag="csT")
                    csT_tiles[blk // 4] = csT_grp

                blk_h2 = []
                s_sb = wp.tile([P, TPB, P], bf16, tag="s_sb")
                for st in range(2):  # supertiles of 4 tiles
                    g0 = st * 4
                    # --- h1T = relu(W1.T @ xT + b1); xT comes straight from the gather
                    h1t_sb = wp.tile([P, 2, 4 * P], f32r, tag="h1t")
                    for m in range(2):
                        h1_ps = ps_big.tile([P, 4 * P], f32, space="PSUM", tag="big")
                        for k in range(3):
                            nc.tensor.matmul(h1_ps[:], lhsT=w1_t[:, k, m, :],
                                             rhs=xt_all[:, k, st * 512:(st + 1) * 512],
                                             start=(k == 0), stop=(k == 2))
                        if m == 0:
                            nc.vector.tensor_scalar(out=h1t_sb[:, m, :], in0=h1_ps[:],
                                                    scalar1=b1_t[:, m:m + 1], scalar2=0.0,
                                                    op0=mybir.AluOpType.add,
                                                    op1=mybir.AluOpType.max)
                        else:
                            nc.scalar.activation(out=h1t_sb[:, m, :], in_=h1_ps[:],
                                                 func=RELU, bias=b1_t[:, m:m + 1])
                    # --- h2 (natural) = relu(h1 @ W2 + b2); 2 tiles per psum
                    h2_sbs = []
                    for pair in range(2):
                        h2_ps = ps_h2.tile([P, 2 * D_H], f32, space="PSUM", tag="h2")
                        for gg in range(2):
                            g = pair * 2 + gg
                            for k2 in range(2):
                                nc.tensor.matmul(
                                    h2_ps[:, gg * D_H:(gg + 1) * D_H],
                                    lhsT=h1t_sb[:, k2, g * P:(g + 1) * P],
                                    rhs=w2_t[:, k2, :],
                                    start=(k2 == 0), stop=(k2 == 1))
                        h2_sb = h2p.tile([P, 2 * D_H], bf16, tag="h2sb")
                        if b2_nonzero:
                            tmp = wp.tile([P, 2 * D_H], f32, tag="h2tmp")
                            for gg in range(2):
                                nc.vector.tensor_tensor(
                                    out=tmp[:, gg * D_H:(gg + 1) * D_H],
                                    in0=h2_ps[:, gg * D_H:(gg + 1) * D_H],
                                    in1=b2_t[:], op=mybir.AluOpType.add)
                            nc.vector.tensor_scalar(out=h2_sb[:], in0=tmp[:],
                                                    scalar1=0.0, scalar2=None,
                                                    op0=mybir.AluOpType.max)
                        else:
                            if pair == 0:
                                nc.vector.tensor_scalar(out=h2_sb[:], in0=h2_ps[:],
                                                        scalar1=0.0, scalar2=None,
                                                        op0=mybir.AluOpType.max)
                            else:
                                nc.scalar.activation(out=h2_sb[:], in_=h2_ps[:],
                                                     func=RELU)
                        h2_sbs.append(h2_sb)
                    blk_h2.extend(h2_sbs)
                    # --- S matrices (gpsimd; Pool engine is mostly idle)
                    s_eng = nc.vector if blk < 2 else nc.gpsimd
                    for g in range(4):
                        t_glob = blk * TPB + g0 + g
                        s_eng.tensor_scalar(out=s_sb[:, g0 + g, :], in0=iota_t[:],
                                            scalar1=ccol_t[:, t_glob:t_glob + 1],
                                            scalar2=None,
                                            op0=mybir.AluOpType.is_equal)
                # segment matmul, emitted directly transposed: csT[f, c] += h2.T @ S.
                # f-chunk-major so each PSUM region's accumulation group is
                # uninterrupted (interleaved regions corrupt accumulation).
                for fc in range(2):
                    for tin in range(TPB):
                        nc.tensor.matmul(
                            cs_ps[:, fc, :],
                            lhsT=blk_h2[tin // 2][:, (tin % 2) * D_H + fc * P:
                                                  (tin % 2) * D_H + (fc + 1) * P],
                            rhs=s_sb[:, tin, :],
                            start=(tin == 0), stop=(tin == TPB - 1))
                # block done -> copy transposed sums into the rho-group tile
                nc.vector.tensor_copy(out=csT_grp[:, :, blk % 4, :], in_=cs_ps[:])

                # --- rho, software-pipelined two blocks behind its group (copies
                # are done, and only the final group remains after the loop)
                if blk >= 5 and (blk - 5) % 4 == 0:
                    grp = (blk - 5) // 4
                    emit_rho(grp, csT_tiles.pop(grp))
            # flush the last pipelined rho group
            emit_rho(B // 4 - 1, csT_tiles.pop(B // 4 - 1))
    nc.compile()
    return nc


_CACHE = {}


def _get_nc(B, NROWS, ph_split, weights_np):
    key = (B, NROWS, ph_split, bool(np.any(weights_np["b2"] != 0.0)))
    if key not in _CACHE:
        _CACHE[key] = _build(B, NROWS, ph_split, weights_np)
    return _CACHE[key]


def kernel(chunk_features, flat_nodes_t, cell_asgn_t, M,
           W1, b1, W2, b2, R1, c1, R2, c2, R3, c3):
    chunk_features = np.asarray(chunk_features, np.float32)
    in_maps, meta = _prepare(chunk_features, np.asarray(flat_nodes_t),
                             np.asarray(cell_asgn_t))
    B, NROWS, ph_split = meta["B"], meta["NROWS"], meta["ph_split"]

    weights_np = {"b2": np.asarray(b2, np.float32)}
    nc = _get_nc(B, NROWS, ph_split, weights_np)

    w_shared = {
        "w1": np.asarray(W1, np.float32).astype(ml_dtypes.bfloat16),
        "w2": np.ascontiguousarray(np.asarray(W2, np.float32)),
        "r1": np.ascontiguousarray(np.asarray(R1, np.float32)),
        "r2": np.ascontiguousarray(np.asarray(R2, np.float32)),
        "r3": np.ascontiguousarray(np.asarray(R3, np.float32)),
        "b1": np.asarray(b1, np.float32).reshape(2, P).T.copy(),
        "c1": np.asarray(c1, np.float32).reshape(2, P).T.copy(),
        "c2": np.asarray(c2, np.float32).reshape(2, P).T.copy(),
        "c3": np.asarray(c3, np.float32).reshape(2, P).T.copy(),
        "iota": np.broadcast_to(np.arange(P, dtype=np.float32)[None, :],
                                (P, P)).copy(),
    }
    if bool(np.any(weights_np["b2"] != 0.0)):
        w_shared["b2row"] = np.broadcast_to(np.asarray(b2, np.float32)[None, :],
                                            (P, D_H)).copy()
    for im in in_maps:
        im.update(w_shared)

    res = run_bass_kernel_spmd(nc, in_maps, core_ids=list(range(NCORES)))

    OUT = np.zeros((M_CELLS, D_OUT), np.float32)
    for c in range(NCORES):
        o = res.results[c]["out"]  # [256, B*128]
        for b, first, ncc in meta["out_maps"][c]:
            OUT[first:first + ncc, :] = o[:, b * P:b * P + ncc].T
    return OUT


# revision 39
# speedup vs baseline: 1.0079x; 1.0079x over previous
"""DeepSet cell encoder on 8 Trainium2 NeuronCores.

Strategy: sort edges by cell, split cells into 8 contiguous ranges with
~equal edge counts (one range per core -> no collectives). Each core:
  - batched dma_gather of node features (bf16, phase-compacted tables so
    indices fit int16)
  - phi MLP: layer1 bf16 (PE-transposed x), layer2 fp32r
  - segment-sum via per-tile one-hot matmul accumulated in PSUM per
    128-cell block
  - rho MLP in transposed layout, fp32r, N=512
Output written transposed [256, cells]; host unpacks to [M, 256].
"""
import sys

if "/opt/trn_rl_repo" not in sys.path:
    sys.path.insert(0, "/opt/trn_rl_repo")

import numpy as np
import ml_dtypes

import concourse.bass as bass
import concourse.mybir as mybir
import concourse.tile as tile
from concourse import bacc
from concourse.bass_utils import run_bass_kernel_spmd
from concourse.masks import make_identity

P = 128
N_NODES, E_TOT, M_CELLS = 100000, 400000, 50000
D_IN, D_H, D_OUT = 384, 256, 256
NCORES = 8
TPB = 8            # tiles per block
BLK_EDGES = TPB * P   # 1024 edge slots per block
NPH = 2            # index phases (int16 range)

f32 = mybir.dt.float32
f32r = mybir.dt.float32r
bf16 = mybir.dt.bfloat16
i16 = mybir.dt.int16
RELU = mybir.ActivationFunctionType.Relu


# ----------------------------------------------------------------- host prep
def _prepare(chunk_features, flat_nodes, cell_asgn):
    x_bf = chunk_features.astype(ml_dtypes.bfloat16)
    cells = np.asarray(cell_asgn, dtype=np.int64)
    nodes = np.asarray(flat_nodes, dtype=np.int64)
    E = cells.shape[0]

    order = np.argsort(cells, kind="stable")
    sc = cells[order]
    sn = nodes[order]
    counts = np.bincount(cells, minlength=M_CELLS)
    cum = np.concatenate([[0], np.cumsum(counts)])  # cum[m] = edges before cell m

    # split cells into NCORES contiguous ranges with ~equal edges
    targets = [E * k // NCORES for k in range(1, NCORES)]
    splits = [0] + [int(np.searchsorted(cum, t)) for t in targets] + [M_CELLS]

    cores = []
    max_blocks = 0
    for c in range(NCORES):
        m_lo, m_hi = splits[c], splits[c + 1]
        blocks = []  # (first_cell, ncells, e_start, e_end)
        m = m_lo
        while m < m_hi:
            first = m
            e0 = cum[m]
            nc_in = 0
            while m < m_hi and nc_in < P and (cum[m + 1] - e0) <= BLK_EDGES:
                m += 1
                nc_in += 1
            if nc_in == 0:
                raise ValueError(f"cell {m} has {counts[m]} edges > {BLK_EDGES}")
            blocks.append((first, nc_in, int(e0), int(cum[m])))
        cores.append({"m_lo": m_lo, "m_hi": m_hi, "blocks": blocks})
        max_blocks = max(max_blocks, len(blocks))

    B = ((max_blocks + 3) // 4) * 4  # multiple of 4 for rho groups
    T = B * TPB

    # per-core slot arrays + phase tables
    ph_split = (B // 2 + 1) // 2 * 2  # blocks in phase 0 (even)
    ph_split = B // 2
    data = []
    nrows = 0
    for c in range(NCORES):
        blocks = cores[c]["blocks"]
        slot_node = np.zeros((T * P,), np.int64)   # node id per slot (pad -> 0)
        slot_ccol = np.full((T * P,), -1.0, np.float32)
        out_map = []  # (block, first_cell, ncells)
        for b in range(B):
            if b < len(blocks):
                first, nc_in, e0, e1 = blocks[b]
                n_e = e1 - e0
                s0 = b * BLK_EDGES
                slot_node[s0:s0 + n_e] = sn[e0:e1]
                slot_ccol[s0:s0 + n_e] = (sc[e0:e1] - first).astype(np.float32)
                out_map.append((b, first, nc_in))
            # else: empty pad block
        # phases
        ph_of_block = np.arange(B) >= ph_split
        idx16 = np.zeros((T * P,), np.int16)
        uniqs = []
        for ph in range(NPH):
            blk_sel = np.where(ph_of_block == bool(ph))[0]
            slots = np.concatenate([np.arange(b * BLK_EDGES, (b + 1) * BLK_EDGES)
                                    for b in blk_sel])
            nd = slot_node[slots]
            real = nd >= 0
            u, inv = np.unique(nd[real], return_inverse=True)
            if len(u) == 0:
                u = np.array([0], np.int64)
            loc = np.zeros(len(slots), np.int64)
            loc[real] = inv
            idx16[slots] = loc.astype(np.int16)
            uniqs.append(u)
            nrows = max(nrows, len(u))
        data.append({"idx16": idx16, "ccol": slot_ccol, "uniqs": uniqs,
                     "out_map": out_map})
    assert nrows <= 32000, f"phase unique rows {nrows} exceed int16 range"
    NROWS = nrows

    # build final per-core input arrays
    in_maps = []
    for c in range(NCORES):
        d = data[c]
        tabs = []
        for ph in range(NPH):
            t = np.zeros((NROWS, D_IN), ml_dtypes.bfloat16)
            u = d["uniqs"][ph]
            t[:len(u)] = x_bf[u]
            tabs.append(t)
        # idx wrap per gather op (= per block): k = t_in_block*128 + p
        # value at [k%16 + 16*rep, k//16] ; 64 cols per block
        idx_blocks = d["idx16"].reshape(B, TPB * P)
        wrap = np.zeros((P, B * 64), np.int16)
        for b in range(B):
            w16 = idx_blocks[b].reshape(64, 16).T  # [16, 64]
            wrap[:, b * 64:(b + 1) * 64] = np.tile(w16, (8, 1))
        ccol = d["ccol"].reshape(T, P).T.copy()  # [128, T] col t = tile t
        in_maps.append({"tab0": tabs[0], "tab1": tabs[1], "gidx": wrap,
                        "gidx0": np.ascontiguousarray(wrap[:, :4 * 64]),
                        "ccol": np.ascontiguousarray(ccol)})
    meta = {"B": B, "T": T, "NROWS": NROWS, "ph_split": ph_split,
            "out_maps": [d["out_map"] for d in data],
            "m_ranges": [(cores[c]["m_lo"], cores[c]["m_hi"]) for c in range(NCORES)]}
    return in_maps, meta


# ------------------------------------------------------------- device build
def _build(B, NROWS, ph_split, weights_np):
    T = B * TPB
    nc = bacc.Bacc()
    tabs = [nc.dram_tensor(f"tab{p}", [NROWS, D_IN], bf16, kind="ExternalInput")
            for p in range(NPH)]
    gidx = nc.dram_tensor("gidx", [P, B * 64], i16, kind="ExternalInput")
    gidx0 = nc.dram_tensor("gidx0", [P, 4 * 64], i16, kind="ExternalInput")
    ccol = nc.dram_tensor("ccol", [P, T], f32, kind="ExternalInput")
    w1 = nc.dram_tensor("w1", [D_IN, D_H], bf16, kind="ExternalInput")
    w2 = nc.dram_tensor("w2", [D_H, D_H], f32r, kind="ExternalInput")
    r1 = nc.dram_tensor("r1", [D_H, D_H], f32r, kind="ExternalInput")
    r2 = nc.dram_tensor("r2", [D_H, D_H], f32r, kind="ExternalInput")
    r3 = nc.dram_tensor("r3", [D_H, D_OUT], f32r, kind="ExternalInput")
    b1d = nc.dram_tensor("b1", [P, 2], f32, kind="ExternalInput")
    c1d = nc.dram_tensor("c1", [P, 2], f32, kind="ExternalInput")
    c2d = nc.dram_tensor("c2", [P, 2], f32, kind="ExternalInput")
    c3d = nc.dram_tensor("c3", [P, 2], f32, kind="ExternalInput")
    iota = nc.dram_tensor("iota", [P, P], f32, kind="ExternalInput")
    out = nc.dram_tensor("out", [D_OUT, B * P], f32, kind="ExternalOutput")

    b2_nonzero = bool(np.any(weights_np["b2"] != 0.0))
    b2d = None
    if b2_nonzero:
        b2d = nc.dram_tensor("b2row", [P, D_H], f32, kind="ExternalInput")

    with tile.TileContext(nc) as tc:
        with (
            tc.tile_pool(name="const", bufs=1) as cpool,
            tc.tile_pool(name="xg", bufs=3) as xgp,
            tc.tile_pool(name="work", bufs=3) as wp,
            tc.tile_pool(name="csb", bufs=6) as csp,
            tc.tile_pool(name="h2p", bufs=6) as h2p,
            tc.tile_pool(name="ps_big", bufs=4, space="PSUM") as ps_big,
            tc.tile_pool(name="ps_h2", bufs=2, space="PSUM") as ps_h2,
            tc.tile_pool(name="ps_cs", bufs=2, space="PSUM") as ps_cs,
        ):
            # ---- constants
            iota_t = cpool.tile([P, P], f32)
            nc.sync.dma_start(iota_t[:], iota[:])
            gidx0_t = cpool.tile([P, 4 * 64], i16)
            nc.sync.dma_start(gidx0_t[:], gidx0[:])
            gidx_t = cpool.tile([P, B * 64], i16)
            nc.sync.dma_start(gidx_t[:], gidx[:])
            ccol_t = cpool.tile([P, T], f32)
            nc.sync.dma_start(ccol_t[:], ccol[:])
            w1_t = cpool.tile([P, 3, 2, P], bf16)
            for k in range(3):
                for m in range(2):
                    nc.sync.dma_start(w1_t[:, k, m, :],
                                      w1[k * P:(k + 1) * P, m * P:(m + 1) * P])
            w2_t = cpool.tile([P, 2, D_H], f32r)
            for k in range(2):
                nc.sync.dma_start(w2_t[:, k, :], w2[k * P:(k + 1) * P, :])
            rts = []
            for name, drt in (("r1", r1), ("r2", r2), ("r3", r3)):
                rt = cpool.tile([P, 2, 2, P], f32r, tag=f"{name}t")
                for k in range(2):
                    for m in range(2):
                        nc.sync.dma_start(rt[:, k, m, :],
                                          drt[k * P:(k + 1) * P, m * P:(m + 1) * P])
                rts.append(rt)
            r1_t, r2_t, r3_t = rts
            b1_t = cpool.tile([P, 2], f32)
            nc.sync.dma_start(b1_t[:], b1d[:])
            c1_t = cpool.tile([P, 2], f32)
            nc.sync.dma_start(c1_t[:], c1d[:])
            c2_t = cpool.tile([P, 2], f32)
            nc.sync.dma_start(c2_t[:], c2d[:])
            c3_t = cpool.tile([P, 2], f32)
            nc.sync.dma_start(c3_t[:], c3d[:])
            b2_t = None
            if b2_nonzero:
                b2_t = cpool.tile([P, D_H], f32)
                nc.sync.dma_start(b2_t[:], b2d[:])

            cs_ps_tiles = {}
            csT_tiles = {}
            xt_tiles = {}

            def issue_gather(b):
                t = xgp.tile([P, 3, BLK_EDGES], bf16, tag="xg")
                nc.gpsimd.dma_gather(
                    out_ap=t[:], in_ap=(tabs[0] if b < ph_split else tabs[1])[:],
                    idxs_ap=(gidx0_t[:, b * 64:(b + 1) * 64] if b < 4
                             else gidx_t[:, b * 64:(b + 1) * 64]),
                    num_idxs=BLK_EDGES, num_idxs_reg=BLK_EDGES,
                    elem_size=D_IN, single_packet=False, transpose=True,
                )
                return t

            def emit_rho(grp, rin):
                rin_slice = lambda t, k: t[:, k, :, :] if len(t.shape) == 4 else t[:, k, :]
                for li, (rt, cb) in enumerate(((r1_t, c1_t), (r2_t, c2_t))):
                    rout = wp.tile([P, 2, 4 * P], f32r, tag=f"r{li}o")
                    for m in range(2):
                        r_ps = ps_big.tile([P, 4 * P], f32, space="PSUM", tag="big")
                        for k in range(2):
                            nc.tensor.matmul(r_ps[:], lhsT=rt[:, k, m, :],
                                             rhs=rin_slice(rin, k),
                                             start=(k == 0), stop=(k == 1))
                        nc.scalar.activation(out=rout[:, m, :], in_=r_ps[:],
                                             func=RELU, bias=cb[:, m:m + 1])
                    rin = rout
                for m in range(2):
                    r_ps = ps_big.tile([P, 4 * P], f32, space="PSUM", tag="big")
                    for k in range(2):
                        nc.tensor.matmul(r_ps[:], lhsT=r3_t[:, k, m, :],
                                         rhs=rin_slice(rin, k),
                                         start=(k == 0), stop=(k == 1))
                    o_sb = wp.tile([P, 4 * P], f32, tag="o_sb")
                    nc.vector.tensor_scalar(out=o_sb[:], in0=r_ps[:],
                                            scalar1=c3_t[:, m:m + 1], scalar2=None,
                                            op0=mybir.AluOpType.add)
                    nc.sync.dma_start(
                        out[m * P:(m + 1) * P, grp * 4 * P:(grp + 1) * 4 * P],
                        o_sb[:])

            for blk in range(B):
                xt_all = issue_gather(blk)
                # csT accumulator: [fchunk, f128, cells] transposed segment sums
                cs_ps = ps_cs.tile([P, 2, P], f32, space="PSUM", tag="cs")
                cs_ps_tiles[blk] = cs_ps
                if blk % 4 == 0:
                    csT_grp = wp.tile([P, 2, 4, P], f32r, tag="csT")
                    csT_tiles[blk // 4] = csT_grp

                blk_h2 = []
                s_sb = wp.tile([P, TPB, P], bf16, tag="s_sb")
                for st in range(2):  # supertiles of 4 tiles
                    g0 = st * 4
                    # --- h1T = relu(W1.T @ xT + b1); xT comes straight from the gather
                    h1t_sb = wp.tile([P, 2, 4 * P], f32r, tag="h1t")
                    for m in range(2):
                        h1_ps = ps_big.tile([P, 4 * P], f32, space="PSUM", tag="big")
                        for k in range(3):
                            nc.tensor.matmul(h1_ps[:], lhsT=w1_t[:, k, m, :],
                                             rhs=xt_all[:, k, st * 512:(st + 1) * 512],
                                             start=(k == 0), stop=(k == 2))
                        if m == 0:
                            nc.vector.tensor_scalar(out=h1t_sb[:, m, :], in0=h1_ps[:],
                                                    scalar1=b1_t[:, m:m + 1], scalar2=0.0,
                                                    op0=mybir.AluOpType.add,
                                                    op1=mybir.AluOpType.max)
                        else:
                            nc.scalar.activation(out=h1t_sb[:, m, :], in_=h1_ps[:],
                                                 func=RELU, bias=b1_t[:, m:m + 1])
                    # --- h2 (natural) = relu(h1 @ W2 + b2); 2 tiles per psum
                    h2_sbs = []
                    for pair in range(2):
                        h2_ps = ps_h2.tile([P, 2 * D_H], f32, space="PSUM", tag="h2")
                        for gg in range(2):
                            g = pair * 2 + gg
                            for k2 in range(2):
                                nc.tensor.matmul(
                                    h2_ps[:, gg * D_H:(gg + 1) * D_H],
                                    lhsT=h1t_sb[:, k2, g * P:(g + 1) * P],
                                    rhs=w2_t[:, k2, :],
                                    start=(k2 == 0), stop=(k2 == 1))
                        h2_sb = h2p.tile([P, 2 * D_H], bf16, tag="h2sb")
                        if b2_nonzero:
                            tmp = wp.tile([P, 2 * D_H], f32, tag="h2tmp")
                            for gg in range(2):
                                nc.vector.tensor_tensor(
                                    out=tmp[:, gg * D_H:(gg + 1) * D_H],
                                    in0=h2_ps[:, gg * D_H:(gg + 1) * D_H],
                                    in1=b2_t[:], op=mybir.AluOpType.add)
                            nc.vector.tensor_scalar(out=h2_sb[:], in0=tmp[:],
                                                    scalar1=0.0, scalar2=None,
                                                    op0=mybir.AluOpType.max)
                        else:
                            if pair == 0:
                                nc.vector.tensor_scalar(out=h2_sb[:], in0=h2_ps[:],
                                                        scalar1=0.0, scalar2=None,
                                                        op0=mybir.AluOpType.max)
                            else:
                                nc.scalar.activation(out=h2_sb[:], in_=h2_ps[:],
                                                     func=RELU)
                        h2_sbs.append(h2_sb)
                    blk_h2.extend(h2_sbs)
                    # --- S matrices (gpsimd; Pool engine is mostly idle)
                    for g in range(4):
                        t_glob = blk * TPB + g0 + g
                        nc.gpsimd.tensor_scalar(out=s_sb[:, g0 + g, :], in0=iota_t[:],
                                                scalar1=ccol_t[:, t_glob:t_glob + 1],
                                                scalar2=None,
                                                op0=mybir.AluOpType.is_equal)
                # segment matmul, emitted directly transposed: csT[f, c] += h2.T @ S.
                # f-chunk-major so each PSUM region's accumulation group is
                # uninterrupted (interleaved regions corrupt accumulation).
                for fc in range(2):
                    for tin in range(TPB):
                        nc.tensor.matmul(
                            cs_ps[:, fc, :],
                            lhsT=blk_h2[tin // 2][:, (tin % 2) * D_H + fc * P:
                                                  (tin % 2) * D_H + (fc + 1) * P],
                            rhs=s_sb[:, tin, :],
                            start=(tin == 0), stop=(tin == TPB - 1))
                # block done -> copy transposed sums into the rho-group tile
                nc.vector.tensor_copy(out=csT_grp[:, :, blk % 4, :], in_=cs_ps[:])

                # --- rho, software-pipelined one block behind (group g emitted
                # after block 4g+7 so its csT copies are long since done)
                grp = blk // 4 - 1
                if blk % 4 == 3 and grp >= 0:
                    emit_rho(grp, csT_tiles.pop(grp))
            # flush the last pipelined rho group
            emit_rho(B // 4 - 1, csT_tiles.pop(B // 4 - 1))
    nc.compile()
    return nc


_CACHE = {}


def _get_nc(B, NROWS, ph_split, weights_np):
    key = (B, NROWS, ph_split, bool(np.any(weights_np["b2"] != 0.0)))
    if key not in _CACHE:
        _CACHE[key] = _build(B, NROWS, ph_split, weights_np)
    return _CACHE[key]


def kernel(chunk_features, flat_nodes_t, cell_asgn_t, M,
           W1, b1, W2, b2, R1, c1, R2, c2, R3, c3):
    chunk_features = np.asarray(chunk_features, np.float32)
    in_maps, meta = _prepare(chunk_features, np.asarray(flat_nodes_t),
                             np.asarray(cell_asgn_t))
    B, NROWS, ph_split = meta["B"], meta["NROWS"], meta["ph_split"]

    weights_np = {"b2": np.asarray(b2, np.float32)}
    nc = _get_nc(B, NROWS, ph_split, weights_np)

    w_shared = {
        "w1": np.asarray(W1, np.float32).astype(ml_dtypes.bfloat16),
        "w2": np.ascontiguousarray(np.asarray(W2, np.float32)),
        "r1": np.ascontiguousarray(np.asarray(R1, np.float32)),
        "r2": np.ascontiguousarray(np.asarray(R2, np.float32)),
        "r3": np.ascontiguousarray(np.asarray(R3, np.float32)),
        "b1": np.asarray(b1, np.float32).reshape(2, P).T.copy(),
        "c1": np.asarray(c1, np.float32).reshape(2, P).T.copy(),
        "c2": np.asarray(c2, np.float32).reshape(2, P).T.copy(),
        "c3": np.asarray(c3, np.float32).reshape(2, P).T.copy(),
        "iota": np.broadcast_to(np.arange(P, dtype=np.float32)[None, :],
                                (P, P)).copy(),
    }
    if bool(np.any(weights_np["b2"] != 0.0)):
        w_shared["b2row"] = np.broadcast_to(np.asarray(b2, np.float32)[None, :],
                                            (P, D_H)).copy()
    for im in in_maps:
        im.update(w_shared)

    res = run_bass_kernel_spmd(nc, in_maps, core_ids=list(range(NCORES)))

    OUT = np.zeros((M_CELLS, D_OUT), np.float32)
    for c in range(NCORES):
        o = res.results[c]["out"]  # [256, B*128]
        for b, first, ncc in meta["out_maps"][c]:
            OUT[first:first + ncc, :] = o[:, b * P:b * P + ncc].T
    return OUT


# revision 43
# speedup vs baseline: 1.0444x; 1.0363x over previous
"""DeepSet cell encoder on 8 Trainium2 NeuronCores.

Strategy: sort edges by cell, split cells into 8 contiguous ranges with
~equal edge counts (one range per core -> no collectives). Each core:
  - batched dma_gather of node features (bf16, phase-compacted tables so
    indices fit int16)
  - phi MLP: layer1 bf16 (PE-transposed x), layer2 fp32r
  - segment-sum via per-tile one-hot matmul accumulated in PSUM per
    128-cell block
  - rho MLP in transposed layout, fp32r, N=512
Output written transposed [256, cells]; host unpacks to [M, 256].
"""
import sys

if "/opt/trn_rl_repo" not in sys.path:
    sys.path.insert(0, "/opt/trn_rl_repo")

import numpy as np
import ml_dtypes

import concourse.bass as bass
import concourse.mybir as mybir
import concourse.tile as tile
from concourse import bacc
from concourse.bass_utils import run_bass_kernel_spmd
from concourse.masks import make_identity

P = 128
N_NODES, E_TOT, M_CELLS = 100000, 400000, 50000
D_IN, D_H, D_OUT = 384, 256, 256
NCORES = 8
TPB = 8            # tiles per block
BLK_EDGES = TPB * P   # 1024 edge slots per block
NPH = 2            # index phases (int16 range)

f32 = mybir.dt.float32
f32r = mybir.dt.float32r
bf16 = mybir.dt.bfloat16
i16 = mybir.dt.int16
RELU = mybir.ActivationFunctionType.Relu


# ----------------------------------------------------------------- host prep
def _prepare(chunk_features, flat_nodes, cell_asgn):
    x_bf = chunk_features.astype(ml_dtypes.bfloat16)
    cells = np.asarray(cell_asgn, dtype=np.int64)
    nodes = np.asarray(flat_nodes, dtype=np.int64)
    E = cells.shape[0]

    order = np.argsort(cells, kind="stable")
    sc = cells[order]
    sn = nodes[order]
    counts = np.bincount(cells, minlength=M_CELLS)
    cum = np.concatenate([[0], np.cumsum(counts)])  # cum[m] = edges before cell m

    # split cells into NCORES contiguous ranges with ~equal edges
    targets = [E * k // NCORES for k in range(1, NCORES)]
    splits = [0] + [int(np.searchsorted(cum, t)) for t in targets] + [M_CELLS]

    cores = []
    max_blocks = 0
    for c in range(NCORES):
        m_lo, m_hi = splits[c], splits[c + 1]
        blocks = []  # (first_cell, ncells, e_start, e_end)
        m = m_lo
        while m < m_hi:
            first = m
            e0 = cum[m]
            nc_in = 0
            while m < m_hi and nc_in < P and (cum[m + 1] - e0) <= BLK_EDGES:
                m += 1
                nc_in += 1
            if nc_in == 0:
                raise ValueError(f"cell {m} has {counts[m]} edges > {BLK_EDGES}")
            blocks.append((first, nc_in, int(e0), int(cum[m])))
        cores.append({"m_lo": m_lo, "m_hi": m_hi, "blocks": blocks})
        max_blocks = max(max_blocks, len(blocks))

    B = max_blocks  # rho handles a partial final group
    T = B * TPB

    # per-core slot arrays + phase tables
    ph_split = (B // 2 + 1) // 2 * 2  # blocks in phase 0 (even)
    ph_split = B // 2
    data = []
    nrows = 0
    for c in range(NCORES):
        blocks = cores[c]["blocks"]
        slot_node = np.zeros((T * P,), np.int64)   # node id per slot (pad -> 0)
        slot_ccol = np.full((T * P,), -1.0, np.float32)
        out_map = []  # (block, first_cell, ncells)
        for b in range(B):
            if b < len(blocks):
                first, nc_in, e0, e1 = blocks[b]
                n_e = e1 - e0
                s0 = b * BLK_EDGES
                slot_node[s0:s0 + n_e] = sn[e0:e1]
                slot_ccol[s0:s0 + n_e] = (sc[e0:e1] - first).astype(np.float32)
                out_map.append((b, first, nc_in))
            # else: empty pad block
        # phases
        ph_of_block = np.arange(B) >= ph_split
        idx16 = np.zeros((T * P,), np.int16)
        uniqs = []
        for ph in range(NPH):
            blk_sel = np.where(ph_of_block == bool(ph))[0]
            if len(blk_sel) == 0:
                uniqs.append(np.array([0], np.int64))
                nrows = max(nrows, 1)
                continue
            slots = np.concatenate([np.arange(b * BLK_EDGES, (b + 1) * BLK_EDGES)
                                    for b in blk_sel])
            nd = slot_node[slots]
            real = nd >= 0
            u, inv = np.unique(nd[real], return_inverse=True)
            if len(u) == 0:
                u = np.array([0], np.int64)
            loc = np.zeros(len(slots), np.int64)
            loc[real] = inv
            idx16[slots] = loc.astype(np.int16)
            uniqs.append(u)
            nrows = max(nrows, len(u))
        data.append({"idx16": idx16, "ccol": slot_ccol, "uniqs": uniqs,
                     "out_map": out_map})
    assert nrows <= 32000, f"phase unique rows {nrows} exceed int16 range"
    NROWS = nrows

    # build final per-core input arrays
    in_maps = []
    for c in range(NCORES):
        d = data[c]
        tabs = []
        for ph in range(NPH):
            t = np.zeros((NROWS, D_IN), ml_dtypes.bfloat16)
            u = d["uniqs"][ph]
            t[:len(u)] = x_bf[u]
            tabs.append(t)
        # idx wrap per gather op (= per block): k = t_in_block*128 + p
        # value at [k%16 + 16*rep, k//16] ; 64 cols per block
        idx_blocks = d["idx16"].reshape(B, TPB * P)
        wrap = np.zeros((P, B * 64), np.int16)
        for b in range(B):
            w16 = idx_blocks[b].reshape(64, 16).T  # [16, 64]
            wrap[:, b * 64:(b + 1) * 64] = np.tile(w16, (8, 1))
        ccol = d["ccol"].reshape(T, P).T.copy()  # [128, T] col t = tile t
        in_maps.append({"tab0": tabs[0], "tab1": tabs[1], "gidx": wrap,
                        "gidx0": np.ascontiguousarray(wrap[:, :min(4, B) * 64]),
                        "ccol": np.ascontiguousarray(ccol)})
    meta = {"B": B, "T": T, "NROWS": NROWS, "ph_split": ph_split,
            "out_maps": [d["out_map"] for d in data],
            "m_ranges": [(cores[c]["m_lo"], cores[c]["m_hi"]) for c in range(NCORES)]}
    return in_maps, meta


# ------------------------------------------------------------- device build
def _build(B, NROWS, ph_split, weights_np):
    T = B * TPB
    nc = bacc.Bacc()
    tabs = [nc.dram_tensor(f"tab{p}", [NROWS, D_IN], bf16, kind="ExternalInput")
            for p in range(NPH)]
    gidx = nc.dram_tensor("gidx", [P, B * 64], i16, kind="ExternalInput")
    G0 = min(4, B)
    gidx0 = nc.dram_tensor("gidx0", [P, G0 * 64], i16, kind="ExternalInput")
    ccol = nc.dram_tensor("ccol", [P, T], f32, kind="ExternalInput")
    w1 = nc.dram_tensor("w1", [D_IN, D_H], bf16, kind="ExternalInput")
    w2 = nc.dram_tensor("w2", [D_H, D_H], f32r, kind="ExternalInput")
    r1 = nc.dram_tensor("r1", [D_H, D_H], f32r, kind="ExternalInput")
    r2 = nc.dram_tensor("r2", [D_H, D_H], f32r, kind="ExternalInput")
    r3 = nc.dram_tensor("r3", [D_H, D_OUT], f32r, kind="ExternalInput")
    b1d = nc.dram_tensor("b1", [P, 2], f32, kind="ExternalInput")
    c1d = nc.dram_tensor("c1", [P, 2], f32, kind="ExternalInput")
    c2d = nc.dram_tensor("c2", [P, 2], f32, kind="ExternalInput")
    c3d = nc.dram_tensor("c3", [P, 2], f32, kind="ExternalInput")
    iota = nc.dram_tensor("iota", [P, P], f32, kind="ExternalInput")
    out = nc.dram_tensor("out", [D_OUT, B * P], f32, kind="ExternalOutput")

    b2_nonzero = bool(np.any(weights_np["b2"] != 0.0))
    b2d = None
    if b2_nonzero:
        b2d = nc.dram_tensor("b2row", [P, D_H], f32, kind="ExternalInput")

    with tile.TileContext(nc) as tc:
        with (
            tc.tile_pool(name="const", bufs=1) as cpool,
            tc.tile_pool(name="xg", bufs=3) as xgp,
            tc.tile_pool(name="work", bufs=3) as wp,
            tc.tile_pool(name="csb", bufs=6) as csp,
            tc.tile_pool(name="h2p", bufs=6) as h2p,
            tc.tile_pool(name="ps_big", bufs=4, space="PSUM") as ps_big,
            tc.tile_pool(name="ps_h2", bufs=2, space="PSUM") as ps_h2,
            tc.tile_pool(name="ps_cs", bufs=2, space="PSUM") as ps_cs,
        ):
            # ---- constants
            iota_t = cpool.tile([P, P], f32)
            nc.sync.dma_start(iota_t[:], iota[:])
            gidx0_t = cpool.tile([P, G0 * 64], i16)
            nc.sync.dma_start(gidx0_t[:], gidx0[:])
            gidx_t = cpool.tile([P, B * 64], i16)
            nc.sync.dma_start(gidx_t[:], gidx[:])
            ccol_t = cpool.tile([P, T], f32)
            nc.sync.dma_start(ccol_t[:], ccol[:])
            w1_t = cpool.tile([P, 3, 2, P], bf16)
            for k in range(3):
                for m in range(2):
                    nc.sync.dma_start(w1_t[:, k, m, :],
                                      w1[k * P:(k + 1) * P, m * P:(m + 1) * P])
            w2_t = cpool.tile([P, 2, D_H], f32r)
            for k in range(2):
                nc.sync.dma_start(w2_t[:, k, :], w2[k * P:(k + 1) * P, :])
            rts = []
            for name, drt in (("r1", r1), ("r2", r2), ("r3", r3)):
                rt = cpool.tile([P, 2, 2, P], f32r, tag=f"{name}t")
                for k in range(2):
                    for m in range(2):
                        nc.sync.dma_start(rt[:, k, m, :],
                                          drt[k * P:(k + 1) * P, m * P:(m + 1) * P])
                rts.append(rt)
            r1_t, r2_t, r3_t = rts
            b1_t = cpool.tile([P, 2], f32)
            nc.sync.dma_start(b1_t[:], b1d[:])
            c1_t = cpool.tile([P, 2], f32)
            nc.sync.dma_start(c1_t[:], c1d[:])
            c2_t = cpool.tile([P, 2], f32)
            nc.sync.dma_start(c2_t[:], c2d[:])
            c3_t = cpool.tile([P, 2], f32)
            nc.sync.dma_start(c3_t[:], c3d[:])
            b2_t = None
            if b2_nonzero:
                b2_t = cpool.tile([P, D_H], f32)
                nc.sync.dma_start(b2_t[:], b2d[:])

            cs_ps_tiles = {}
            csT_tiles = {}
            xt_tiles = {}

            def issue_gather(b):
                t = xgp.tile([P, 3, BLK_EDGES], bf16, tag="xg")
                nc.gpsimd.dma_gather(
                    out_ap=t[:], in_ap=(tabs[0] if b < ph_split else tabs[1])[:],
                    idxs_ap=(gidx0_t[:, b * 64:(b + 1) * 64] if b < G0
                             else gidx_t[:, b * 64:(b + 1) * 64]),
                    num_idxs=BLK_EDGES, num_idxs_reg=BLK_EDGES,
                    elem_size=D_IN, single_packet=False, transpose=True,
                )
                return t

            def emit_rho(grp, rin, gsz):
                N = gsz * P
                rin_slice = lambda t, k: t[:, k, :gsz, :] if len(t.shape) == 4 else t[:, k, :N]
                for li, (rt, cb) in enumerate(((r1_t, c1_t), (r2_t, c2_t))):
                    rout = wp.tile([P, 2, 4 * P], f32r, tag=f"r{li}o")
                    for m in range(2):
                        r_ps = ps_big.tile([P, 4 * P], f32, space="PSUM", tag="big")
                        for k in range(2):
                            nc.tensor.matmul(r_ps[:, :N], lhsT=rt[:, k, m, :],
                                             rhs=rin_slice(rin, k),
                                             start=(k == 0), stop=(k == 1))
                        nc.scalar.activation(out=rout[:, m, :N], in_=r_ps[:, :N],
                                             func=RELU, bias=cb[:, m:m + 1])
                    rin = rout
                for m in range(2):
                    r_ps = ps_big.tile([P, 4 * P], f32, space="PSUM", tag="big")
                    for k in range(2):
                        nc.tensor.matmul(r_ps[:, :N], lhsT=r3_t[:, k, m, :],
                                         rhs=rin[:, k, :N],
                                         start=(k == 0), stop=(k == 1))
                    o_sb = wp.tile([P, 4 * P], f32, tag="o_sb")
                    nc.vector.tensor_scalar(out=o_sb[:, :N], in0=r_ps[:, :N],
                                            scalar1=c3_t[:, m:m + 1], scalar2=None,
                                            op0=mybir.AluOpType.add)
                    nc.sync.dma_start(
                        out[m * P:(m + 1) * P, grp * 4 * P:grp * 4 * P + N],
                        o_sb[:, :N])

            for blk in range(B):
                xt_all = issue_gather(blk)
                # csT accumulator: [fchunk, f128, cells] transposed segment sums
                cs_ps = ps_cs.tile([P, 2, P], f32, space="PSUM", tag="cs")
                cs_ps_tiles[blk] = cs_ps
                if blk % 4 == 0:
                    csT_grp = wp.tile([P, 2, 4, P], f32r, tag="csT")
                    csT_tiles[blk // 4] = csT_grp

                blk_h2 = []
                s_sb = wp.tile([P, TPB, P], bf16, tag="s_sb")
                for st in range(2):  # supertiles of 4 tiles
                    g0 = st * 4
                    # --- h1T = relu(W1.T @ xT + b1); xT comes straight from the gather
                    h1t_sb = wp.tile([P, 2, 4 * P], f32r, tag="h1t")
                    for m in range(2):
                        h1_ps = ps_big.tile([P, 4 * P], f32, space="PSUM", tag="big")
                        for k in range(3):
                            nc.tensor.matmul(h1_ps[:], lhsT=w1_t[:, k, m, :],
                                             rhs=xt_all[:, k, st * 512:(st + 1) * 512],
                                             start=(k == 0), stop=(k == 2))
                        if m == 0:
                            nc.vector.tensor_scalar(out=h1t_sb[:, m, :], in0=h1_ps[:],
                                                    scalar1=b1_t[:, m:m + 1], scalar2=0.0,
                                                    op0=mybir.AluOpType.add,
                                                    op1=mybir.AluOpType.max)
                        else:
                            nc.scalar.activation(out=h1t_sb[:, m, :], in_=h1_ps[:],
                                                 func=RELU, bias=b1_t[:, m:m + 1])
                    # --- h2 (natural) = relu(h1 @ W2 + b2); 2 tiles per psum
                    h2_sbs = []
                    for pair in range(2):
                        h2_ps = ps_h2.tile([P, 2 * D_H], f32, space="PSUM", tag="h2")
                        for gg in range(2):
                            g = pair * 2 + gg
                            for k2 in range(2):
                                nc.tensor.matmul(
                                    h2_ps[:, gg * D_H:(gg + 1) * D_H],
                                    lhsT=h1t_sb[:, k2, g * P:(g + 1) * P],
                                    rhs=w2_t[:, k2, :],
                                    start=(k2 == 0), stop=(k2 == 1))
                        h2_sb = h2p.tile([P, 2 * D_H], bf16, tag="h2sb")
                        if b2_nonzero:
                            tmp = wp.tile([P, 2 * D_H], f32, tag="h2tmp")
                            for gg in range(2):
                                nc.vector.tensor_tensor(
                                    out=tmp[:, gg * D_H:(gg + 1) * D_H],
                                    in0=h2_ps[:, gg * D_H:(gg + 1) * D_H],
                                    in1=b2_t[:], op=mybir.AluOpType.add)
                            nc.vector.tensor_scalar(out=h2_sb[:], in0=tmp[:],
                                                    scalar1=0.0, scalar2=None,
                                                    op0=mybir.AluOpType.max)
                        else:
                            if pair == 0:
                                nc.vector.tensor_scalar(out=h2_sb[:], in0=h2_ps[:],
                                                        scalar1=0.0, scalar2=None,
                                                        op0=mybir.AluOpType.max)
                            else:
                                nc.scalar.activation(out=h2_sb[:], in_=h2_ps[:],
                                                     func=RELU)
                        h2_sbs.append(h2_sb)
                    blk_h2.extend(h2_sbs)
                    # --- S matrices (gpsimd; Pool engine is mostly idle)
                    for g in range(4):
                        t_glob = blk * TPB + g0 + g
                        nc.gpsimd.tensor_scalar(out=s_sb[:, g0 + g, :], in0=iota_t[:],
                                                scalar1=ccol_t[:, t_glob:t_glob + 1],
                                                scalar2=None,
                                                op0=mybir.AluOpType.is_equal)
                # segment matmul, emitted directly transposed: csT[f, c] += h2.T @ S.
                # f-chunk-major so each PSUM region's accumulation group is
                # uninterrupted (interleaved regions corrupt accumulation).
                for fc in range(2):
                    for tin in range(TPB):
                        nc.tensor.matmul(
                            cs_ps[:, fc, :],
                            lhsT=blk_h2[tin // 2][:, (tin % 2) * D_H + fc * P:
                                                  (tin % 2) * D_H + (fc + 1) * P],
                            rhs=s_sb[:, tin, :],
                            start=(tin == 0), stop=(tin == TPB - 1))
                # block done -> copy transposed sums into the rho-group tile
                nc.vector.tensor_copy(out=csT_grp[:, :, blk % 4, :], in_=cs_ps[:])

                # --- rho, software-pipelined one block behind (group g emitted
                # after block 4g+7 so its csT copies are long since done)
                grp = blk // 4 - 1
                if blk % 4 == 3 and grp >= 0:
                    emit_rho(grp, csT_tiles.pop(grp), 4)
            # flush remaining pipelined rho groups (last may be partial)
            for grp in sorted(csT_tiles):
                emit_rho(grp, csT_tiles.pop(grp), min(4, B - 4 * grp))
    nc.compile()
    return nc


_CACHE = {}


def _get_nc(B, NROWS, ph_split, weights_np):
    key = (B, NROWS, ph_split, bool(np.any(weights_np["b2"] != 0.0)))
    if key not in _CACHE:
        _CACHE[key] = _build(B, NROWS, ph_split, weights_np)
    return _CACHE[key]


def kernel(chunk_features, flat_nodes_t, cell_asgn_t, M,
           W1, b1, W2, b2, R1, c1, R2, c2, R3, c3):
    chunk_features = np.asarray(chunk_features, np.float32)
    in_maps, meta = _prepare(chunk_features, np.asarray(flat_nodes_t),
                             np.asarray(cell_asgn_t))
    B, NROWS, ph_split = meta["B"], meta["NROWS"], meta["ph_split"]

    weights_np = {"b2": np.asarray(b2, np.float32)}
    nc = _get_nc(B, NROWS, ph_split, weights_np)

    w_shared = {
        "w1": np.asarray(W1, np.float32).astype(ml_dtypes.bfloat16),
        "w2": np.ascontiguousarray(np.asarray(W2, np.float32)),
        "r1": np.ascontiguousarray(np.asarray(R1, np.float32)),
        "r2": np.ascontiguousarray(np.asarray(R2, np.float32)),
        "r3": np.ascontiguousarray(np.asarray(R3, np.float32)),
        "b1": np.asarray(b1, np.float32).reshape(2, P).T.copy(),
        "c1": np.asarray(c1, np.float32).reshape(2, P).T.copy(),
        "c2": np.asarray(c2, np.float32).reshape(2, P).T.copy(),
        "c3": np.asarray(c3, np.float32).reshape(2, P).T.copy(),
        "iota": np.broadcast_to(np.arange(P, dtype=np.float32)[None, :],
                                (P, P)).copy(),
    }
    if bool(np.any(weights_np["b2"] != 0.0)):
        w_shared["b2row"] = np.broadcast_to(np.asarray(b2, np.float32)[None, :],
                                            (P, D_H)).copy()
    for im in in_maps:
        im.update(w_shared)

    res = run_bass_kernel_spmd(nc, in_maps, core_ids=list(range(NCORES)))

    OUT = np.zeros((M_CELLS, D_OUT), np.float32)
    for c in range(NCORES):
        o = res.results[c]["out"]  # [256, B*128]
        for b, first, ncc in meta["out_maps"][c]:
            OUT[first:first + ncc, :] = o[:, b * P:b * P + ncc].T
    return OUT
